# revision 9
# baseline (speedup 1.0000x reference)
"""Trainium2 Bass kernel for AdditiveMSSDLoss.

Computes, over B samples:
  pos_err = ||pred_position - target_position|| / diameter
  rot_err = 2 * max_radius * sin(theta/2) / diameter,
     where theta is the relative rotation angle between the two quaternions.
Returns (mean(pos_err + rot_err), mean(pos_err), mean(rot_err)).

Key algebraic identity used on-device: for quaternions p, q (unnormalized),
  trace(R(p̂) R(q̂)ᵀ) = 4 d² - 1   with  d = (p·q) / (|p||q|)
  cos θ = 2 d² - 1,  sin(θ/2) = sqrt(max(0, 1 - d²))
so  rot_err = 2 * max_radius * sqrt(max(0, u - v) / u) / diameter
with u = (p·p)(q·q), v = (p·q)².  No arccos/sin/3x3 matrices needed.

Sharding: pure data-parallel over 8 NeuronCores; each core reduces its
524288-sample shard to per-partition partial sums; the host sums the
8 x [128, 2T] partials in float64 and divides by B.
"""

import numpy as np

import concourse.bass as bass
import concourse.tile as tile
from concourse import bacc, mybir
from concourse.bass_utils import run_bass_kernel_spmd

B = 4194304
M = 8                     # NeuronCores
NPC = B // M              # samples per core = 524288
P = 128                   # SBUF partitions
WTOT = NPC // P           # samples per partition = 4096
W = 512                   # samples per partition per tile
T = WTOT // W             # tiles = 8

F32 = mybir.dt.float32
AF = mybir.ActivationFunctionType
OP = mybir.AluOpType

_CACHE = {}
LAST_EXEC_NS = None


def _build(npc=NPC, w=W):
    T = npc // (P * w)
    W = w
    nc = bacc.Bacc("TRN2", target_bir_lowering=False, debug=False, num_devices=M)

    d_pp = nc.declare_dram_parameter("pred_position", [npc, 3], F32, isOutput=False)
    d_pr = nc.declare_dram_parameter("pred_rotation", [npc, 4], F32, isOutput=False)
    d_tp = nc.declare_dram_parameter("target_position", [npc, 3], F32, isOutput=False)
    d_tr = nc.declare_dram_parameter("target_rotation", [npc, 4], F32, isOutput=False)
    d_mr = nc.declare_dram_parameter("max_radius", [npc], F32, isOutput=False)
    d_di = nc.declare_dram_parameter("diameter", [npc], F32, isOutput=False)
    d_out = nc.declare_dram_parameter("out", [P, 2 * T], F32, isOutput=True)

    # sample s = t*(P*W) + p*W + w  ->  tile t, partition p, free w
    v_pp = d_pp[:, :].rearrange("(t p w) c -> t p (w c)", t=T, p=P, w=W)
    v_tp = d_tp[:, :].rearrange("(t p w) c -> t p (w c)", t=T, p=P, w=W)
    v_pr = d_pr[:, :].rearrange("(t p w) c -> t p (w c)", t=T, p=P, w=W)
    v_tr = d_tr[:, :].rearrange("(t p w) c -> t p (w c)", t=T, p=P, w=W)
    v_mr = d_mr[:].rearrange("(t p w) -> t p w", t=T, p=P, w=W)
    v_di = d_di[:].rearrange("(t p w) -> t p w", t=T, p=P, w=W)

    with tile.TileContext(nc) as tc:
        with (
            tc.tile_pool(name="io", bufs=2) as io,
            tc.tile_pool(name="tmp", bufs=2) as tmp,
            tc.tile_pool(name="acc", bufs=1) as acc,
        ):
            parts = acc.tile([P, 2 * T], F32)  # [:, :T]=pos sums, [:, T:]=rot sums

            for t in range(T):
                t_pos = io.tile([P, 6 * W], F32, tag="pos")   # [pp | tp]
                t_rot = io.tile([P, 8 * W], F32, tag="rot")   # [pr | tr]
                t_md = io.tile([P, 2 * W], F32, tag="md")     # [mr | di]
                nc.gpsimd.dma_start(out=t_pos[:, : 3 * W], in_=v_pp[t])
                nc.gpsimd.dma_start(out=t_pos[:, 3 * W :], in_=v_tp[t])
                nc.gpsimd.dma_start(out=t_rot[:, : 4 * W], in_=v_pr[t])
                nc.gpsimd.dma_start(out=t_rot[:, 4 * W :], in_=v_tr[t])
                nc.gpsimd.dma_start(out=t_md[:, :W], in_=v_mr[t])
                nc.gpsimd.dma_start(out=t_md[:, W:], in_=v_di[t])

                # ---- position: pos2 = sum_c (pp_c - tp_c)^2 ----
                dp = tmp.tile([P, 3 * W], F32, tag="dp")
                nc.vector.tensor_sub(dp[:, :], t_pos[:, : 3 * W], t_pos[:, 3 * W :])
                nc.scalar.square(dp[:, :], dp[:, :])          # dp := dp^2
                d3 = dp[:, :].rearrange("p (w c) -> p w c", c=3)
                pos2 = tmp.tile([P, W], F32, tag="pos2")
                nc.vector.tensor_add(pos2[:, :], d3[:, :, 0], d3[:, :, 1])
                nc.vector.tensor_add(pos2[:, :], pos2[:, :], d3[:, :, 2])

                # ---- rotation dots: rq = [p*p | q*q | p*q] products ----
                rq = tmp.tile([P, 12 * W], F32, tag="rq")
                nc.scalar.square(rq[:, : 8 * W], t_rot[:, :])
                nc.vector.tensor_mul(
                    rq[:, 8 * W :], t_rot[:, : 4 * W], t_rot[:, 4 * W :]
                )
                r2 = rq[:, :].rearrange("p (w c) -> p w c", c=2)
                h = tmp.tile([P, 6 * W], F32, tag="h")
                nc.vector.tensor_add(h[:, :], r2[:, :, 0], r2[:, :, 1])
                h2 = h[:, :].rearrange("p (w c) -> p w c", c=2)
                dots = tmp.tile([P, 3 * W], F32, tag="dots")
                nc.vector.tensor_add(dots[:, :], h2[:, :, 0], h2[:, :, 1])
                ppd = dots[:, 0:W]
                qqd = dots[:, W : 2 * W]
                pqd = dots[:, 2 * W : 3 * W]

                # ---- scalar chain ----
                u = tmp.tile([P, W], F32, tag="u")
                nc.vector.tensor_mul(u[:, :], ppd, qqd)       # u = pp*qq
                v = tmp.tile([P, W], F32, tag="v")
                nc.scalar.square(v[:, :], pqd)                # v = pq^2
                nc.vector.tensor_sub(v[:, :], u[:, :], v[:, :])   # v := u - v
                nc.vector.tensor_scalar_max(v[:, :], v[:, :], 0.0)  # v := w
                d2 = tmp.tile([P, W], F32, tag="d2")
                nc.scalar.square(d2[:, :], t_md[:, W:])       # d2 = di^2
                nc.vector.tensor_mul(d2[:, :], d2[:, :], u[:, :])  # d2 := z = d2*u
                rz = tmp.tile([P, W], F32, tag="rz")
                nc.vector.reciprocal_approx_fast(out=rz[:, :], in_=d2[:, :])
                nc.vector.tensor_mul(u[:, :], rz[:, :], u[:, :])   # u := rec2 = 1/di^2
                nc.vector.tensor_mul(v[:, :], v[:, :], rz[:, :])   # v := a = w/(d2*u)
                nc.scalar.activation(v[:, :], v[:, :], AF.Sqrt, scale=4.0)  # v := 2*sqrt(a)
                scr = tmp.tile([P, W], F32, tag="scr")
                nc.vector.scalar_tensor_tensor(
                    out=scr[:, :],
                    in0=t_md[:, :W],                          # mr
                    scalar=1.0,
                    in1=v[:, :],                              # 2*sqrt(a)
                    op0=OP.mult,
                    op1=OP.mult,
                    accum_out=parts[:, T + t : T + t + 1],
                )
                nc.vector.tensor_mul(pos2[:, :], pos2[:, :], u[:, :])  # pos2 := b
                nc.scalar.activation(
                    pos2[:, :], pos2[:, :], AF.Sqrt,
                    accum_out=parts[:, t : t + 1],
                )

            nc.gpsimd.dma_start(out=d_out[:, :], in_=parts[:, :])

    nc.compile()
    return nc


def kernel(pred_position, pred_rotation, target_position, target_rotation,
           max_radius, diameter):
    global LAST_EXEC_NS
    if "nc" not in _CACHE:
        _CACHE["nc"] = _build()
    nc = _CACHE["nc"]

    def shard(x):
        x = np.ascontiguousarray(np.asarray(x, dtype=np.float32))
        return [x[i * NPC : (i + 1) * NPC] for i in range(M)]

    ins = {
        "pred_position": shard(pred_position),
        "pred_rotation": shard(pred_rotation),
        "target_position": shard(target_position),
        "target_rotation": shard(target_rotation),
        "max_radius": shard(max_radius),
        "diameter": shard(diameter),
    }
    in_maps = [{k: ins[k][i] for k in ins} for i in range(M)]

    res = run_bass_kernel_spmd(nc, in_maps, core_ids=list(range(M)))
    LAST_EXEC_NS = res.exec_time_ns

    pos_sum = 0.0
    rot_sum = 0.0
    for i in range(M):
        o = res.results[i]["out"].astype(np.float64)
        pos_sum += o[:, :T].sum()
        rot_sum += o[:, T:].sum()
    pos_mean = pos_sum / B
    rot_mean = rot_sum / B
    return (
        np.float32(pos_mean + rot_mean),
        np.float32(pos_mean),
        np.float32(rot_mean),
    )
